# revision 1
# baseline (speedup 1.0000x reference)
"""NetVLAD forward on 8 Trainium2 NeuronCores.

Full inputs: x [16, 128, 64, 64] f32, conv_w [64, 128], conv_b [64],
centroids [64, 128]. Output [16, 8192] f32.

Sharding: data-parallel over batch — 2 samples per core; weights replicated.

Per-sample math (C=128 channels, N=4096 positions, K=64 clusters):
  r[n]   = 1/||x[:, n]||                    (channel L2 norm)
  logits = (conv_w @ x) * r[n] + b          (1x1 conv on normalized x)
  a      = softmax_k(logits)
  vlad   = sum_n a[k,n] * (x[:,n]*r[n]) - centroids[k] * sum_n a[k,n]
  out    = rownorm(vlad) / sqrt(K)          (global norm == sqrt(K) exactly
                                             since rows are unit after intra)

Layout strategy per core:
  - x loaded naturally [C=128 part, N free]; mm1 produces logits [K, N].
  - PE transposes move x chunks and logit chunks into n-partitioned layout
    where softmax reduces along the free dim and the VLAD GEMM contracts n.
  - Scale folding: with es = exp(r*l0)*exp(b) (unnormalized softmax numer)
    and rs = 1/sum_k es, the VLAD matmul uses lhsT = es directly and
    rhs = [x_t*(r*rs) | x_t*(r*rs) | rs | 1] so no separate "a" tensor is
    ever materialized; the rs column yields A_k = sum_n a[k,n].
  - ACT activation-table discipline: only Sqrt and Exp are used (rsqrt =
    Sqrt(reciprocal), reciprocal on DVE), ordered to load each table once.
"""

import os

import numpy as np

import concourse.bass as bass
import concourse.bacc as bacc
import concourse.tile as tile
from concourse import mybir
from concourse.bass_utils import run_bass_kernel_spmd
from concourse.masks import make_identity

f32 = mybir.dt.float32
f32r = mybir.dt.float32r
f16 = mybir.dt.float16
AF = mybir.ActivationFunctionType
ALU = mybir.AluOpType
AX = mybir.AxisListType

B, C, N, K = 16, 128, 4096, 64
NCORES = 8
BS = B // NCORES          # samples per core = 2
GRP = 512                 # n per mm1 group
CH = 128                  # n per chunk
NGRP = N // GRP           # 8
NCH = GRP // CH           # 4 chunks per group

# PE dtype for the x-side pipeline (mm1, x transposes, mm2):
#   f32r: 1 cyc/row at FD>=256, ~1.6e-4 matmul error
#   f32:  exact, 4 cyc/row
PE_DT = {"f32r": f32r, "f32": f32}[os.environ.get("PE_DT", "f32r")]


def _bcast_free(ap, n, total_free):
    """AP view of [P, F] tile replicated n times along a middle free dim."""
    return bass.AP(tensor=ap.tensor, offset=ap.offset,
                   ap=[list(ap.ap[0]), [0, n], [1, total_free]])


def _build():
    nc = bacc.Bacc("TRN2", target_bir_lowering=False, debug=False,
                   num_devices=NCORES)
    x_h = nc.dram_tensor("x", [BS, C, N], f32, kind="ExternalInput")
    w_h = nc.dram_tensor("conv_w", [K, C], f32, kind="ExternalInput")
    b_h = nc.dram_tensor("conv_b", [K], f32, kind="ExternalInput")
    c_h = nc.dram_tensor("centroids", [K, C], f32, kind="ExternalInput")
    o_h = nc.dram_tensor("out", [BS, K * C], f32, kind="ExternalOutput")

    with tile.TileContext(nc) as tc:
        _emit(nc, tc, x_h, w_h, b_h, c_h, o_h)
    nc.compile()
    return nc


def _emit(nc, tc, x_h, w_h, b_h, c_h, o_h):
    import contextlib
    ctx = contextlib.ExitStack()
    with ctx:
        const = ctx.enter_context(tc.tile_pool(name="const", bufs=1))
        sqp = ctx.enter_context(tc.tile_pool(name="sqp", bufs=2))
        l0p = ctx.enter_context(tc.tile_pool(name="l0p", bufs=3))
        e0p = ctx.enter_context(tc.tile_pool(name="e0p", bufs=10))
        esp = ctx.enter_context(tc.tile_pool(name="esp", bufs=3))
        vec = ctx.enter_context(tc.tile_pool(name="vec", bufs=6))
        fin = ctx.enter_context(tc.tile_pool(name="fin", bufs=4))
        ps_l0 = ctx.enter_context(tc.tile_pool(name="ps_l0", bufs=2, space="PSUM"))
        ps_t = ctx.enter_context(tc.tile_pool(name="ps_t", bufs=4, space="PSUM"))
        ps_v = ctx.enter_context(tc.tile_pool(name="ps_v", bufs=1, space="PSUM"))

        # ---- constants ----
        ident = const.tile([128, 128], f32, tag="ident")
        make_identity(nc, ident[:])
        id_r = const.tile([128, 128], f32r, tag="id_r")
        nc.vector.tensor_copy(out=id_r[:], in_=ident[:])
        if PE_DT is f32r:
            id_x = id_r
        else:
            id_x = ident

        w_sb = const.tile([K, C], f32, tag="w_sb")
        nc.sync.dma_start(out=w_sb[:], in_=w_h[:, :])
        ps_wt = ps_t.tile([128, K], f32, tag="pt")
        nc.tensor.transpose(ps_wt[:], w_sb[:], ident[0:K, 0:K])
        w_t = const.tile([C, K], PE_DT, tag="w_t")
        nc.vector.tensor_copy(out=w_t[:], in_=ps_wt[:])

        b_ap = b_h[:]
        b_bcast = bass.AP(tensor=b_ap.tensor, offset=b_ap.offset,
                          ap=[[0, 128], [1, K]])
        b_rep = const.tile([128, K], f32, tag="b_rep")
        nc.gpsimd.dma_start(out=b_rep[:], in_=b_bcast)
        expb = const.tile([128, K], f16, tag="expb")

        cent = const.tile([K, C], f32, tag="cent")
        nc.sync.dma_start(out=cent[:], in_=c_h[:, :])

        ones_f32 = const.tile([128, 1], f32, tag="ones")
        nc.vector.memset(ones_f32[:], 1.0)

        # persistent mm2-rhs tiles [xn0 | xn1 | rs0 | rs1], manual rotation
        NROT = 3
        xts = []
        for t in range(NROT):
            xt = const.tile([128, 264], PE_DT, tag=f"xtp{t}")
            xts.append(xt)

        ps_vlad = ps_v.tile([128, 264], f32, tag="vlad")

        # ---- load all of x up front (2MB/sample, f32r cast in DMA) ----
        x_sb = []
        for s in range(BS):
            xt_ = const.tile([128, N], PE_DT, tag=f"xsb{s}")
            for h in range(2):
                sl = slice(h * (N // 2), (h + 1) * (N // 2))
                if PE_DT is f32r:
                    nc.gpsimd.dma_start(out=xt_[:, sl], in_=x_h[s, :, sl])
                else:
                    nc.sync.dma_start(out=xt_[:, sl], in_=x_h[s, :, sl])
            x_sb.append(xt_)

        # ---- channel norms for the whole input ----
        # ns[128, s, ci] = sum_c x^2 ; r = 1/sqrt(ns) via DVE recip + ACT Sqrt
        ns_all = const.tile([128, BS, N // CH], f32, tag="ns_all")
        for s in range(BS):
            for h in range(2):
                sq16 = sqp.tile([128, N // 2], f16, tag="sq",
                                name=f"sq_{s}_{h}")
                sl = slice(h * (N // 2), (h + 1) * (N // 2))
                nc.gpsimd.tensor_mul(out=sq16[:],
                                     in0=x_sb[s][:, sl].bitcast(f32),
                                     in1=x_sb[s][:, sl].bitcast(f32))
                nc.vector.tensor_reduce(
                    out=ns_all[:, s, h * 16:(h + 1) * 16],
                    in_=sq16[:].rearrange("p (g c) -> p g c", c=CH),
                    axis=AX.X, op=ALU.add)
        u_all = const.tile([128, BS, N // CH], f32, tag="u_all")
        nc.vector.reciprocal(out=u_all[:], in_=ns_all[:])
        r_all = const.tile([128, BS, N // CH], f32, tag="r_all")
        nc.scalar.activation(out=r_all[:], in_=u_all[:], func=AF.Sqrt)

        # expb after the Sqrt so the ACT table sequence is Sqrt->Exp...->Sqrt
        nc.scalar.activation(out=expb[:], in_=b_rep[:], func=AF.Exp)

        # ---- main loop over 512-wide groups ----
        for g in range(NGRP):
            # mm1 per sample; pack logits into one [128, 512] sbuf tile
            l0_sb = l0p.tile([128, GRP], f32r, tag="l0sb")
            for s in range(BS):
                pl0 = ps_l0.tile([K, GRP], f32, tag="l0",
                                 name=f"pl0_{g}_{s}")
                nc.tensor.matmul(
                    pl0[:], w_t[:], x_sb[s][:, g * GRP:(g + 1) * GRP],
                    start=True, stop=True)
                if (g + s) % 2 == 0:
                    nc.vector.tensor_copy(
                        out=l0_sb[s * K:(s + 1) * K, :], in_=pl0[:])
                else:
                    nc.scalar.activation(
                        out=l0_sb[s * K:(s + 1) * K, :], in_=pl0[:],
                        func=AF.Copy)

            es_g = esp.tile([128, NCH, BS, K], PE_DT, tag="es",
                            name=f"es_{g}")
            for j in range(NCH):
                ci = g * NCH + j
                # transposes: logits chunk (both samples ride along) + x
                plt = ps_t.tile([128, 128], f32r, tag="pt",
                                name=f"plt_{g}_{j}")
                nc.tensor.transpose(
                    plt[:], l0_sb[:, j * CH:(j + 1) * CH], id_r[:])
                e0 = e0p.tile([128, BS, K], f16, tag="e0",
                              name=f"e0_{g}_{j}")
                for s in range(BS):
                    nc.scalar.activation(
                        out=e0[:, s, :],
                        in_=plt[:, s * K:(s + 1) * K].bitcast(f32),
                        func=AF.Exp, scale=r_all[:, s, ci:ci + 1])
                # es = e0 * exp(b): one DVE op per chunk (expb broadcast)
                nc.vector.tensor_mul(
                    out=es_g[:, j, :, :], in0=e0[:],
                    in1=_bcast_free(expb[:], BS, K))

            # softmax denominators and fold factors for the group
            ssum = vec.tile([128, BS, NCH], f32, tag="ssum")
            for s in range(BS):
                nc.vector.tensor_reduce(
                    out=ssum[:, s, :], in_=es_g[:, :, s, :],
                    axis=AX.X, op=ALU.add)
            rs_g = vec.tile([128, BS, NCH], f32, tag="rs")
            nc.vector.reciprocal(out=rs_g[:], in_=ssum[:])
            rs_r = vec.tile([128, BS, NCH], PE_DT, tag="rs_r")
            nc.vector.tensor_copy(out=rs_r[:], in_=rs_g[:])
            comb = vec.tile([128, BS, NCH], f32, tag="comb")
            nc.vector.tensor_mul(out=comb[:], in0=rs_g[:],
                                 in1=r_all[:, :, g * NCH:(g + 1) * NCH])

            for j in range(NCH):
                ci = g * NCH + j
                xt_tile = xts[ci % NROT]
                for s in range(BS):
                    pxt = ps_t.tile([128, 128], PE_DT, tag="pt",
                                    name=f"pxt_{g}_{j}_{s}")
                    nc.tensor.transpose(
                        pxt[:], x_sb[s][:, ci * CH:(ci + 1) * CH], id_x[:])
                    # xn'' = x_t * (r*rs): psum->sbuf copy with fold
                    dst = xt_tile[:, s * 128:(s + 1) * 128]
                    cj = comb[:, s, j:j + 1]
                    if (j + s) % 2 == 0:
                        nc.scalar.activation(out=dst, in_=pxt[:].bitcast(f32),
                                             func=AF.Copy, scale=cj)
                    else:
                        nc.vector.tensor_scalar(
                            out=dst, in0=pxt[:].bitcast(f32), scalar1=cj,
                            scalar2=None, op0=ALU.mult)
                # rs columns for the A_k sums (one per sample: the column
                # multiplies every output row, so each sample gets its own;
                # the cross terms land in unused psum cells)
                nc.vector.tensor_copy(out=xt_tile[:, 256:257],
                                      in_=rs_r[:, 0, j:j + 1])
                nc.vector.tensor_copy(out=xt_tile[:, 257:258],
                                      in_=rs_r[:, 1, j:j + 1])
                nc.tensor.matmul(
                    ps_vlad[:, 0:258], es_g[:, j, :, :], xt_tile[:, 0:258],
                    start=(ci == 0), stop=(ci == N // CH - 1))

        # ---- finalize: vlad -> centroid subtract -> rownorm -> out ----
        for s in range(BS):
            vsl = ps_vlad[s * K:(s + 1) * K, s * 128:s * 128 + 128]
            a_col = ps_vlad[s * K:(s + 1) * K, 256 + s:257 + s]
            t1 = fin.tile([K, C], f32, tag="t1")
            nc.vector.tensor_scalar(out=t1[:], in0=cent[:], scalar1=a_col,
                                    scalar2=None, op0=ALU.mult)
            t2 = fin.tile([K, C], f32, tag="t2")
            nc.vector.tensor_sub(out=t2[:], in0=vsl, in1=t1[:])
            sq2 = fin.tile([K, C], f32, tag="sq2")
            nc.vector.tensor_mul(out=sq2[:], in0=t2[:], in1=t2[:])
            rowns = fin.tile([K, 1], f32, tag="rowns")
            nc.vector.tensor_reduce(out=rowns[:], in_=sq2[:], axis=AX.X,
                                    op=ALU.add)
            u2 = fin.tile([K, 1], f32, tag="u2")
            nc.vector.reciprocal(out=u2[:], in_=rowns[:])
            rn = fin.tile([K, 1], f32, tag="rn")
            # 1/(8*sqrt(rowns)) = sqrt((1/64) * (1/rowns))
            nc.scalar.activation(out=rn[:], in_=u2[:], func=AF.Sqrt,
                                 scale=1.0 / 64.0)
            o_sb = fin.tile([K, C], f32, tag="osb")
            nc.vector.tensor_scalar(out=o_sb[:], in0=t2[:], scalar1=rn[:],
                                    scalar2=None, op0=ALU.mult)
            nc.sync.dma_start(
                out=o_h[s, :].rearrange("(k c) -> k c", c=C), in_=o_sb[:])


_NC = None


def kernel(x, conv_w, conv_b, centroids):
    global _NC
    if _NC is None:
        _NC = _build()
    x = np.ascontiguousarray(np.asarray(x, dtype=np.float32)).reshape(B, C, N)
    conv_w = np.asarray(conv_w, dtype=np.float32)
    conv_b = np.asarray(conv_b, dtype=np.float32)
    centroids = np.asarray(centroids, dtype=np.float32)
    in_maps = [{
        "x": x[i * BS:(i + 1) * BS],
        "conv_w": conv_w,
        "conv_b": conv_b,
        "centroids": centroids,
    } for i in range(NCORES)]
    res = run_bass_kernel_spmd(_NC, in_maps, core_ids=list(range(NCORES)))
    return np.concatenate([res.results[i]["out"] for i in range(NCORES)],
                          axis=0)



# revision 7
# speedup vs baseline: 1.4246x; 1.4246x over previous
"""NetVLAD forward on 8 Trainium2 NeuronCores — "flipped" fp16 design.

Full inputs: x [16, 128, 64, 64] f32, conv_w [64, 128], conv_b [64],
centroids [64, 128]. Output [16, 8192] f32.

Sharding: data-parallel over batch — 2 samples per core; weights replicated.

Per-sample math (C=128 channels, N=4096 positions, K=64 clusters):
  r[n]   = 1/||x[:, n]||                    (channel L2 norm)
  l      = (conv_w @ x) * r[n]              (logits on normalized x)
  es     = exp(l) ; es' = es * exp(b)       (softmax numerator)
  S[n]   = sum_k es'[n,k] ; rs = 1/S
  vlad   = sum_n es'[n,k] * (x[:,n] * r[n]*rs[n]) - centroids[k]*A[k]
  out    = rownorm(vlad) / 8                (global norm == 8 exactly)

Structure per core (BS=2 samples):
  - x DMA-cast to fp16 [C=128, N] per sample (gpsimd SWDGE), x^2 split
    GPSIMD/DVE.
  - Per chunk ci (128 positions, 32 per sample): the x-chunk is the PE
    STATIONARY (fp16 -> FWL); one matmul streams [W_t | I] -> psum
    [lt(64) | xt(128)]: transposed logits AND transposed x in one pass.
    A second stationary (x^2 chunk) streams a ones column -> ns[n]
    (channel norms), n-partitioned.
  - r = exp(-0.5*ln(ns)) on ACT: every rsqrt via ln/exp so the whole kernel
    uses ONE activation table set (natural_log_exp_and_others).
  - exp(lt, scale=r) PSUM->SBUF fp16 = es (the psum move comes free).
  - DVE tensor_tensor_reduce: es' = es * expb  AND  S = row-sum, one op.
  - DVE tensor_tensor pair-copy: xtc = xt_psum * comb (comb = r*rs,
    free-stride-0 broadcast AP), f32 psum -> fp16 sbuf.
  - mm2: stationary es' [n, (s,k)], moving [xtc_s0|xtc_s1] (256) + rs cols
    (2) accumulated into one psum [128, 258] over all 32 chunks.
  - rs/comb at minibatch (4-chunk) granularity so only 5 ab-psum tiles are
    ever live (8-bank budget).
"""

import contextlib

import numpy as np

import concourse.bass as bass
import concourse.bacc as bacc
import concourse.tile as tile
from concourse import mybir
from concourse.bass_utils import run_bass_kernel_spmd
from concourse.masks import make_identity

f32 = mybir.dt.float32
f16 = mybir.dt.float16
AF = mybir.ActivationFunctionType
ALU = mybir.AluOpType
AX = mybir.AxisListType

B, C, N, K = 16, 128, 4096, 64
NCORES = 8
BS = B // NCORES          # samples per core = 2
CH = 128                  # positions per chunk
NCH = N // CH             # 32 chunks per sample
NB = 4                    # r batches (ns -> ln -> exp granularity)
CPB = NCH // NB           # 8 chunks per r-batch
MB = 4                    # rs/comb minibatch (psum residency bound)
LN8 = float(np.log(8.0))


def _build():
    nc = bacc.Bacc("TRN2", target_bir_lowering=False, debug=False,
                   num_devices=NCORES)
    x_h = nc.dram_tensor("x", [BS, C, N], f32, kind="ExternalInput")
    w_h = nc.dram_tensor("conv_w", [K, C], f32, kind="ExternalInput")
    b_h = nc.dram_tensor("conv_b", [K], f32, kind="ExternalInput")
    c_h = nc.dram_tensor("centroids", [K, C], f32, kind="ExternalInput")
    o_h = nc.dram_tensor("out", [BS, K * C], f32, kind="ExternalOutput")

    with tile.TileContext(nc) as tc:
        _emit(nc, tc, x_h, w_h, b_h, c_h, o_h)
    nc.compile()
    return nc


def _emit(nc, tc, x_h, w_h, b_h, c_h, o_h):
    ctx = contextlib.ExitStack()
    with ctx:
        const = ctx.enter_context(tc.tile_pool(name="const", bufs=1))
        esp = ctx.enter_context(tc.tile_pool(name="esp", bufs=6))
        xtp = ctx.enter_context(tc.tile_pool(name="xtp", bufs=4))
        sml = ctx.enter_context(tc.tile_pool(name="sml", bufs=3))
        fin = ctx.enter_context(tc.tile_pool(name="fin", bufs=4))
        ps_ab = ctx.enter_context(tc.tile_pool(name="ps_ab", bufs=5,
                                               space="PSUM"))
        ps_ns = ctx.enter_context(tc.tile_pool(name="ps_ns", bufs=2,
                                               space="PSUM"))
        ps_v = ctx.enter_context(tc.tile_pool(name="ps_v", bufs=1,
                                              space="PSUM"))

        # ---- constants ----
        ident = const.tile([128, 128], f32, tag="ident")
        make_identity(nc, ident[:])

        # combined mm1 moving operand [w_t | I] in fp16
        rhs_wi = const.tile([C, K + C], f16, tag="rhs_wi")
        w_sb = sml.tile([K, C], f32, tag="w_sb")
        nc.sync.dma_start(out=w_sb[:], in_=w_h[:, :])
        ps_wt = ps_ns.tile([128, K], f32, tag="ns", name="ps_wt")
        nc.tensor.transpose(ps_wt[:], w_sb[:], ident[0:K, 0:K])
        nc.vector.tensor_copy(out=rhs_wi[:, 0:K], in_=ps_wt[:])
        nc.vector.tensor_copy(out=rhs_wi[:, K:K + C], in_=ident[:])

        ones_col = const.tile([C, 1], f16, tag="ones_col")
        nc.vector.memset(ones_col[:], 1.0)

        # b replicated across partitions (DRAM bcast DMA), then exp(b) fp16
        b_ap = b_h[:]
        b_bc = bass.AP(tensor=b_ap.tensor, offset=b_ap.offset,
                       ap=[[0, 128], [1, K]])
        b_rep = const.tile([128, K], f32, tag="b_rep")
        nc.gpsimd.dma_start(out=b_rep[:], in_=b_bc)
        expb = const.tile([128, K], f16, tag="expb")
        nc.scalar.activation(out=expb[:], in_=b_rep[:], func=AF.Exp)

        cent = const.tile([K, C], f32, tag="cent")
        nc.sync.dma_start(out=cent[:], in_=c_h[:, :])

        # per-(chunk,sample) column tables, col = 2*ci + s
        r_all = const.tile([128, 2 * NCH], f32, tag="r_all")
        s_all = const.tile([128, 2 * NCH], f32, tag="s_all")
        comb_all = const.tile([128, 2 * NCH], f32, tag="comb_all")
        rs16_all = const.tile([128, 2 * NCH], f16, tag="rs16_all")

        # ---- x load (fp16 cast DMA, quarter pieces, sample-interleaved) ----
        x_sb = [const.tile([C, N], f16, tag=f"xsb{s}", name=f"xsb{s}")
                for s in range(BS)]
        xsq = [const.tile([C, N], f16, tag=f"xsq{s}", name=f"xsq{s}")
               for s in range(BS)]
        NQ = 4
        for q in range(NQ):
            sl = slice(q * (N // NQ), (q + 1) * (N // NQ))
            for s in range(BS):
                nc.gpsimd.dma_start(out=x_sb[s][:, sl], in_=x_h[s, :, sl])
        # x^2: early pieces on GPSIMD (idle engine), late ones on DVE
        for q in range(NQ):
            sl = slice(q * (N // NQ), (q + 1) * (N // NQ))
            for s in range(BS):
                if q < 3:
                    nc.gpsimd.tensor_mul(out=xsq[s][:, sl],
                                         in0=x_sb[s][:, sl],
                                         in1=x_sb[s][:, sl])
                else:
                    nc.vector.tensor_mul(out=xsq[s][:, sl],
                                         in0=x_sb[s][:, sl],
                                         in1=x_sb[s][:, sl])

        ps_vlad = ps_v.tile([128, 258], f32, tag="vlad")

        # ---- main loop ----
        ab_t = {}
        esn_t = {}
        for batch in range(NB):
            ns_ps = ps_ns.tile([128, 2 * CPB], f32, tag="ns",
                               name=f"ns_{batch}")
            # ns matmuls for the whole batch (PE, n-partitioned output)
            for cl in range(CPB):
                ci = batch * CPB + cl
                for s in range(BS):
                    nc.tensor.matmul(
                        ns_ps[:, 2 * cl + s:2 * cl + s + 1],
                        xsq[s][:, ci * CH:(ci + 1) * CH],
                        ones_col[:], start=True, stop=True)
            # r = exp(-0.5 * ln(ns))
            lnt = sml.tile([128, 2 * CPB], f32, tag="lnt",
                           name=f"lnt_{batch}")
            nc.scalar.activation(out=lnt[:], in_=ns_ps[:], func=AF.Ln)
            bsl = slice(2 * batch * CPB, 2 * (batch + 1) * CPB)
            nc.scalar.activation(out=r_all[:, bsl], in_=lnt[:],
                                 func=AF.Exp, scale=-0.5)

            for cl in range(CPB):
                ci = batch * CPB + cl
                # psum [lt0 | xt0 | lt1 | xt1]
                ab = ps_ab.tile([128, 2 * (K + C)], f32, tag="ab",
                                name=f"ab_{ci}")
                ab_t[ci] = ab
                for s in range(BS):
                    off = s * (K + C)
                    nc.tensor.matmul(
                        ab[:, off:off + K + C],
                        x_sb[s][:, ci * CH:(ci + 1) * CH],
                        rhs_wi[:], start=True, stop=True)

                es = esp.tile([128, 2, K], f16, tag="es", name=f"es_{ci}")
                esn = esp.tile([128, 2 * K], f16, tag="esn",
                               name=f"esn_{ci}")
                esn_t[ci] = esn
                for s in range(BS):
                    off = s * (K + C)
                    col = 2 * ci + s
                    # es = exp(r * lt): PSUM -> SBUF move comes free
                    nc.scalar.activation(
                        out=es[:, s, :], in_=ab[:, off:off + K],
                        func=AF.Exp, scale=r_all[:, col:col + 1])
                    # es' = es * expb ; S = row-sum — one fused DVE op
                    nc.vector.scalar_tensor_tensor(
                        out=esn[:, s * K:(s + 1) * K], in0=es[:, s, :],
                        scalar=1.0, in1=expb[:], op0=ALU.mult,
                        op1=ALU.mult, accum_out=s_all[:, col:col + 1])

                if ci % MB == MB - 1:
                    # minibatch tail: rs = 1/S, comb = rs*r, rs -> fp16
                    mb0 = ci - (MB - 1)
                    msl = slice(2 * mb0, 2 * (ci + 1))
                    rs_b = sml.tile([128, 2 * MB], f32, tag="rs",
                                    name=f"rs_{ci}")
                    nc.vector.reciprocal(out=rs_b[:], in_=s_all[:, msl])
                    nc.vector.tensor_mul(out=comb_all[:, msl], in0=rs_b[:],
                                         in1=r_all[:, msl])
                    nc.vector.tensor_copy(out=rs16_all[:, msl], in_=rs_b[:])

                    for cj in range(mb0, ci + 1):
                        abj = ab_t.pop(cj)
                        esnj = esn_t.pop(cj)
                        xtc = xtp.tile([128, 2, C], f16, tag="xtc",
                                       name=f"xtc_{cj}")
                        # xtc[:, s, :] = xt_psum_s * comb[:, 2cj+s]
                        xv = abj[:].rearrange("p (a b) -> p a b", b=K + C)
                        xv = xv[:, :, K:K + C]
                        cb = comb_all[:, 2 * cj:2 * cj + 2]
                        cbv = bass.AP(tensor=cb.tensor, offset=cb.offset,
                                      ap=[list(cb.ap[0]), [1, 2], [0, C]])
                        nc.vector.tensor_mul(out=xtc[:], in0=xv, in1=cbv)
                        # mm2: stationary es', moving [xtc(256) | rs(2)]
                        nc.tensor.matmul(
                            ps_vlad[:, 0:2 * C], esnj[:],
                            xtc[:].rearrange("p a b -> p (a b)"),
                            start=(cj == 0), stop=(cj == NCH - 1))
                        nc.tensor.matmul(
                            ps_vlad[:, 2 * C:2 * C + 2], esnj[:],
                            rs16_all[:, 2 * cj:2 * cj + 2],
                            start=(cj == 0), stop=(cj == NCH - 1))

        # ---- finalize: centroid subtract -> rownorm -> /8 -> out ----
        for s in range(BS):
            vsl = ps_vlad[s * K:(s + 1) * K, s * C:(s + 1) * C]
            a_col = ps_vlad[s * K:(s + 1) * K, 2 * C + s:2 * C + s + 1]
            a_sb = fin.tile([K, 1], f32, tag="a_sb", name=f"a_{s}")
            nc.vector.tensor_copy(out=a_sb[:], in_=a_col)
            t1 = fin.tile([K, C], f32, tag="t1", name=f"t1_{s}")
            nc.vector.tensor_scalar(out=t1[:], in0=cent[:], scalar1=a_sb[:],
                                    scalar2=None, op0=ALU.mult)
            t2 = fin.tile([K, C], f32, tag="t2", name=f"t2_{s}")
            nc.vector.tensor_sub(out=t2[:], in0=vsl, in1=t1[:])
            # rowns = sum(t2^2) fused with the square
            sq2 = fin.tile([K, C], f32, tag="sq2", name=f"sq2_{s}")
            rowns = fin.tile([K, 1], f32, tag="rowns", name=f"rns_{s}")
            nc.vector.scalar_tensor_tensor(
                out=sq2[:], in0=t2[:], scalar=1.0, in1=t2[:],
                op0=ALU.mult, op1=ALU.mult, accum_out=rowns[:])
            # rn = exp(-0.5*ln(rowns)) = 1/sqrt(rowns); /8 folded below
            lnr = fin.tile([K, 1], f32, tag="lnr", name=f"lnr_{s}")
            nc.scalar.activation(out=lnr[:], in_=rowns[:], func=AF.Ln)
            rn = fin.tile([K, 1], f32, tag="rn", name=f"rn_{s}")
            nc.scalar.activation(out=rn[:], in_=lnr[:], func=AF.Exp,
                                 scale=-0.5)
            o_sb = fin.tile([K, C], f32, tag="osb", name=f"osb_{s}")
            nc.vector.tensor_scalar(out=o_sb[:], in0=t2[:], scalar1=rn[:],
                                    scalar2=0.125, op0=ALU.mult,
                                    op1=ALU.mult)
            nc.sync.dma_start(
                out=o_h[s, :].rearrange("(k c) -> k c", c=C), in_=o_sb[:])


_NC = None


def kernel(x, conv_w, conv_b, centroids):
    global _NC
    if _NC is None:
        _NC = _build()
    x = np.ascontiguousarray(np.asarray(x, dtype=np.float32)).reshape(B, C, N)
    conv_w = np.asarray(conv_w, dtype=np.float32)
    conv_b = np.asarray(conv_b, dtype=np.float32)
    centroids = np.asarray(centroids, dtype=np.float32)
    in_maps = [{
        "x": x[i * BS:(i + 1) * BS],
        "conv_w": conv_w,
        "conv_b": conv_b,
        "centroids": centroids,
    } for i in range(NCORES)]
    res = run_bass_kernel_spmd(_NC, in_maps, core_ids=list(range(NCORES)))
    return np.concatenate([res.results[i]["out"] for i in range(NCORES)],
                          axis=0)
